# revision 13
# baseline (speedup 1.0000x reference)
"""Trainium2 Bass kernel for nn_DotAttention (B=4, Tq=Tv=2048, D=1024, 16 heads).

Sharding (load-balanced): core c -> head pair (2c, 2c+1) = att-dim slice
[128c, 128c+128) for ALL 4 batches. Attention cost scales with the per-batch
masked length (NJ_b = ceil(len_b/128) Tv tiles of the energy/softmax/context
loop), so giving every core the same head slice of every batch equalizes
work across cores exactly (sum_b NJ_b tile-units each) instead of the
previous batch-sharded layout where the worst core paid NJ_max * 4.

Each core computes q/k projections for its 128 att dims, v for its 2 heads,
masked-softmax attention in transposed-energy layout, and a partial final
projection y_b = ctx_slice @ Wf[slice, :]. Host sums the 8 partials per
batch and adds the bias constant (bv @ Wf + bf, exact because attention
weights sum to 1).

Layouts (SBUF is [128 partitions, free]):
  qT/ctxT [128, 4, T]  partition = att-dim slice, free = (batch, time)
  kT      [128, sum_b TV_b]  concatenated per-batch padded Tv extents
  v       [128, sum_b NJ_b, 130]  per head 65 cols (64 v + ones col)
  energy^T in PSUM [128(Tv), 2*512]: both heads side by side
  ctx^T in PSUM [65, 512] per head; row 64 = softmax denominator (ones col)

All matmuls run in bf16 (fp32 PSUM accumulation). The program is
specialized on the tuple of per-batch NJ values; emission order interleaves
next-batch projection work and final-projection units ("fillers") into the
ACT(exp)-bound attention stream so the PE never idles. Batches run in
descending-NJ order so long batches hide later batches' projections.
"""

import sys

sys.path.insert(0, "/opt/trn_rl_repo")

import numpy as np
import ml_dtypes

import concourse.bacc as bacc
import concourse.tile as tile
import concourse.mybir as mybir
from concourse.bass_utils import run_bass_kernel_spmd

F32 = mybir.dt.float32
BF16 = mybir.dt.bfloat16
F16 = mybir.dt.float16
MMDT = BF16
MM_NP = ml_dtypes.bfloat16
OUT_NP = np.float16
AF = mybir.ActivationFunctionType

B, T, D, ATT = 4, 2048, 1024, 1024
NH, DH = 16, 64
CD = 128  # att-dim slice per core (2 heads)
NCORES = 8
LARGE = 1e30
SW = 512  # time-span width per streamed input chunk

_cache = {}


def build_nc(NJS, phases="ABC", loop_n=1, splice=True, ebufs=2, pcybufs=3,
             bcast="gpsimd"):
    """NJS: tuple of per-batch NJ (Tv tiles of 128), in BATCH order."""
    NJS = tuple(int(x) for x in NJS)
    key = (NJS, phases, loop_n, splice, ebufs, pcybufs, bcast)
    if key in _cache:
        return _cache[key]
    NSV = [(nj + 3) // 4 for nj in NJS]  # 512-spans of Tv per batch
    TV = [s * SW for s in NSV]
    OFF = np.cumsum([0] + TV).tolist()  # kT/xv col offsets per batch
    JOFF = np.cumsum([0] + list(NJS)).tolist()  # v/mask tile offsets
    TVT, NJT = OFF[B], JOFF[B]
    # process batches in descending NJ (ties by index) for splicing overlap
    border = sorted(range(B), key=lambda b: -NJS[b])

    nc = bacc.Bacc("TRN2", target_bir_lowering=False, debug=False,
                   num_devices=NCORES)

    xq_d = nc.dram_tensor("xq", [B, D, T], MMDT, kind="ExternalInput")
    xv_d = nc.dram_tensor("xv", [D, TVT], MMDT, kind="ExternalInput")
    wq_d = nc.dram_tensor("wq", [D, CD], MMDT, kind="ExternalInput")
    wk_d = nc.dram_tensor("wk", [D, CD], MMDT, kind="ExternalInput")
    wv_d = nc.dram_tensor("wv", [D, 130], MMDT, kind="ExternalInput")
    wf_d = nc.dram_tensor("wf", [CD, ATT], MMDT, kind="ExternalInput")
    mask_d = nc.dram_tensor("mask", [128, NJT], F32, kind="ExternalInput")
    bqk_d = nc.dram_tensor("bqk", [128, 2 * B], F32, kind="ExternalInput")
    y_d = nc.dram_tensor("y", [B, T, ATT], F16, kind="ExternalOutput")

    xv_r = xv_d[:, :].rearrange("(kc p) n -> p kc n", p=128)  # [128, 8, TVT]
    wq_r = wq_d[:, :].rearrange("(kc p) m -> p kc m", p=128)  # [128, 8, 128]
    wk_r = wk_d[:, :].rearrange("(kc p) m -> p kc m", p=128)
    wv_r = wv_d[:, :].rearrange("(kc p) m -> p kc m", p=128)  # [128, 8, 130]

    with tile.TileContext(nc) as tc:
        from contextlib import ExitStack
        _st = ExitStack()
        if loop_n > 1:
            _st.enter_context(tc.For_i(0, loop_n, 1))
        with _st, tc.tile_pool(name="persist", bufs=1) as persist:
            qT = persist.tile([128, B, T], MMDT)
            kT = persist.tile([128, TVT], MMDT)
            v = persist.tile([128, NJT, 130], MMDT)
            ctxT = persist.tile([128, B, T], MMDT)
            wfs = persist.tile([128, ATT], MMDT)
            wqs = persist.tile([128, 8, CD], MMDT)
            wks = persist.tile([128, 8, CD], MMDT)
            wvs = persist.tile([128, 8, 130], MMDT)
            mask = persist.tile([128, NJT], F32)
            bqk = persist.tile([128, 2 * B], F32)

            with (
                tc.tile_pool(name="chunks", bufs=3) as chunks,
                tc.tile_pool(name="expp", bufs=3) as expp,
                tc.tile_pool(name="workp", bufs=4) as workp,
                tc.tile_pool(name="yp", bufs=4) as yp,
                tc.tile_pool(name="rsd", bufs=4, space="DRAM") as rsd,
                tc.tile_pool(name="ppa", bufs=1, space="PSUM") as ppa,
                tc.tile_pool(name="pe", bufs=ebufs, space="PSUM") as pe_pool,
                tc.tile_pool(name="pcy", bufs=pcybufs, space="PSUM") as pcy,
            ):
                # ---------- filler units (emitted into attention slack) ----
                fillers = []  # list of (kind, est_ns, closure)

                def pop_fillers(budget_ns):
                    while fillers and budget_ns > 0:
                        _, est, fn = fillers.pop(0)
                        fn()
                        budget_ns -= est

                def drain_proj(b):
                    # all of batch b's projection units MUST be emitted before
                    # batch b's attention reads qT/kT/v: emission-order defines
                    # the dependency graph, so a later-emitted write is a race.
                    while any(k == ("proj", b) for k, _, _ in fillers):
                        _, _, fn = fillers.pop(0)
                        fn()

                # ---------- projection unit builders ----------------------
                def dma_weights():
                    for kc in range(8):
                        nc.scalar.dma_start(out=wks[:, kc, :], in_=wk_r[:, kc, :])
                    for kc in range(8):
                        nc.scalar.dma_start(out=wvs[:, kc, :], in_=wv_r[:, kc, :])
                    for kc in range(8):
                        nc.scalar.dma_start(out=wqs[:, kc, :], in_=wq_r[:, kc, :])
                    nc.scalar.dma_start(out=mask, in_=mask_d[:, :])
                    nc.scalar.dma_start(out=bqk, in_=bqk_d[:, :])
                    nc.scalar.dma_start(
                        out=wfs,
                        in_=wf_d[:, :].rearrange("(kc p) n -> p kc n", p=128
                                                 )[:, 0, :])
                    # ones columns of v (persist; projection copies skip them)
                    vh = v[:, :, :].rearrange("p j (h x) -> p j h x", h=2, x=65)
                    nc.vector.memset(vh[:, :, :, 64:65], 1.0)

                def emit_xv_chunk(b, s):
                    xc = chunks.tile([128, 8, SW], MMDT, tag="xc",
                                     name=f"xv_{b}_{s}")
                    sl = slice(OFF[b] + s * SW, OFF[b] + (s + 1) * SW)
                    nc.sync.dma_start(out=xc, in_=xv_r[:, :, sl])
                    return xc

                def emit_xq_chunk(b, s):
                    xc = chunks.tile([128, 8, SW], MMDT, tag="xc",
                                     name=f"xq_{b}_{s}")
                    xq_rb = xq_d[b, :, :].rearrange("(kc p) n -> p kc n", p=128)
                    nc.sync.dma_start(out=xc,
                                      in_=xq_rb[:, :, s * SW:(s + 1) * SW])
                    return xc

                def emit_qk_half(xc, ps_cell, wt, dst, bias_col, tag, half):
                    # one K- or Q-projection span split in two 4-mm halves
                    # sharing a single PSUM accumulation group
                    if half == 0:
                        ps_cell[0] = ppa.tile([128, SW], F32, tag="pa",
                                              name=tag)
                    ps = ps_cell[0]
                    for kc in range(half * 4, half * 4 + 4):
                        nc.tensor.matmul(ps[:, :], lhsT=wt[:, kc, :],
                                         rhs=xc[:, kc, :],
                                         start=(kc == 0), stop=(kc == 7))
                    if half == 1:
                        with nc.allow_low_precision(reason="qk store"):
                            nc.vector.tensor_scalar_add(dst, ps[:, :], bias_col)

                def emit_v_unit(xc, b, s, jt):
                    j = s * 4 + jt
                    if j >= NJS[b]:
                        return
                    ps = ppa.tile([128, 512], F32, tag="pa", name=f"v_{b}_{j}")
                    for kc in range(8):
                        nc.tensor.matmul(ps[:, 0:130],
                                         lhsT=xc[:, kc, jt * 128:(jt + 1) * 128],
                                         rhs=wvs[:, kc, :],
                                         start=(kc == 0), stop=(kc == 7))
                    # copy the two 64-wide head slices, skip the ones columns
                    psh = ps[:, 0:130].rearrange("p (h x) -> p h x", x=65)
                    vj = v[:, JOFF[b] + j, :].rearrange("p (h x) -> p h x", x=65)
                    with nc.allow_low_precision(reason="v store"):
                        nc.vector.tensor_copy(out=vj[:, :, 0:64],
                                              in_=psh[:, :, 0:64])

                def proj_units(b):
                    """Filler units for batch b's projections, in dep order."""
                    units = []
                    kind = ("proj", b)
                    for s in range(NSV[b]):
                        xc = [None]

                        def load(b=b, s=s, xc=xc):
                            xc[0] = emit_xv_chunk(b, s)
                        units.append((kind, 150, load))
                        kps = [None]
                        dst = kT[:, OFF[b] + s * SW:OFF[b] + (s + 1) * SW]
                        for half in range(2):
                            units.append(
                                (kind, 900, lambda xc=xc, kps=kps, dst=dst,
                                 b=b, s=s, h=half:
                                 emit_qk_half(xc[0], kps, wks, dst,
                                              bqk[:, B + b:B + b + 1],
                                              f"k_{b}_{s}", h)))
                        for jt in range(4):
                            units.append((kind, 470,
                                          lambda b=b, s=s, jt=jt, xc=xc:
                                          emit_v_unit(xc[0], b, s, jt)))
                    for s in range(4):
                        xc = [None]

                        def load(b=b, s=s, xc=xc):
                            xc[0] = emit_xq_chunk(b, s)
                        units.append((kind, 150, load))
                        qps = [None]
                        dst = qT[:, b, s * SW:(s + 1) * SW]
                        for half in range(2):
                            units.append(
                                (kind, 900, lambda xc=xc, qps=qps, dst=dst,
                                 b=b, s=s, h=half:
                                 emit_qk_half(xc[0], qps, wqs, dst,
                                              bqk[:, b:b + 1],
                                              f"q_{b}_{s}", h)))
                    return units

                def emit_c_unit(b, i, n):
                    y_ps = ppa.tile([128, 512], F32, tag="pa",
                                    name=f"y_{b}_{i}_{n}")
                    nc.tensor.matmul(y_ps[:, :],
                                     lhsT=ctxT[:, b, i * 128:(i + 1) * 128],
                                     rhs=wfs[:, n * 512:(n + 1) * 512],
                                     start=True, stop=True)
                    y_sb = yp.tile([128, 512], F16, tag="ysb")
                    with nc.allow_low_precision(reason="y f16"):
                        nc.vector.tensor_copy(out=y_sb[:, :], in_=y_ps[:, :])
                    nc.gpsimd.dma_start(
                        out=y_d[b, i * 128:(i + 1) * 128,
                                n * 512:(n + 1) * 512],
                        in_=y_sb[:, :])

                # ---------- emission ---------------------------------------
                dma_weights()
                if "A" in phases:
                    for _, _, fn in proj_units(border[0]):
                        fn()

                for bi, b in enumerate(border):
                    if bi + 1 < B and "A" in phases:
                        fillers.extend(proj_units(border[bi + 1]))
                        if not splice:
                            pop_fillers(1e9)
                    drain_proj(b)
                    NJ = NJS[b]
                    for ib in (range(4) if "B" in phases else []):
                        ibs = slice(ib * 512, (ib + 1) * 512)
                        ctxA = pcy.tile([65, 512], F32, tag="cy")
                        ctxB = pcy.tile([65, 512], F32, tag="cy")
                        ctx_ps = (ctxA[:, :], ctxB[:, :])
                        jlist = list(range(NJ))
                        pairs = [jlist[i:i + 2] for i in range(0, NJ, 2)]
                        for jp in pairs:
                            exs = []
                            for j in jp:
                                e_ps = pe_pool.tile([128, 1024], F32, tag="e")
                                for hh in range(2):
                                    p0 = hh * 64
                                    nc.tensor.matmul(
                                        e_ps[:, hh * 512:(hh + 1) * 512],
                                        lhsT=kT[p0:p0 + 64,
                                                OFF[b] + j * 128:
                                                OFF[b] + (j + 1) * 128],
                                        rhs=qT[p0:p0 + 64, b, ibs],
                                        start=True, stop=True,
                                    )
                                ex = expp.tile([128, 1024], MMDT, tag="ex")
                                nc.scalar.activation(
                                    out=ex[:, :], in_=e_ps[:, :], func=AF.Exp,
                                    bias=mask[:, JOFF[b] + j:JOFF[b] + j + 1],
                                    scale=1.0)
                                exs.append(ex)
                            if splice:
                                pop_fillers(500 * len(jp))
                            for j, ex in zip(jp, exs):
                                for hh in range(2):
                                    nc.tensor.matmul(
                                        ctx_ps[hh],
                                        lhsT=v[:, JOFF[b] + j,
                                               hh * 65:(hh + 1) * 65],
                                        rhs=ex[:, hh * 512:(hh + 1) * 512],
                                        start=(j == 0), stop=(j == NJ - 1),
                                    )
                        for hh in range(2):
                            p0 = hh * 64
                            rs = workp.tile([1, 512], F32, tag="rs")
                            nc.vector.reciprocal(out=rs[:, :],
                                                 in_=ctx_ps[hh][64:65, :])
                            bc_sb = workp.tile([64, 512], F32, tag="bcs")
                            if bcast == "gpsimd":
                                nc.gpsimd.partition_broadcast(
                                    bc_sb[:, :], rs[:, :], channels=64)
                            else:
                                rs_dr = rsd.tile([1, 512], F32, tag="rsd")
                                nc.sync.dma_start(out=rs_dr[:, :], in_=rs[:, :])
                                nc.sync.dma_start(
                                    out=bc_sb[:, :],
                                    in_=rs_dr[0:1, :].partition_broadcast(64))
                            with nc.allow_low_precision(reason="ctx store"):
                                nc.vector.tensor_mul(
                                    ctxT[p0:p0 + 64, b, ibs],
                                    ctx_ps[hh][0:64, :], bc_sb[:, :])
                        if "C" in phases:
                            for i in range(ib * 4, ib * 4 + 4):
                                for n in range(2):
                                    fillers.append(
                                        ("c", 400, lambda b=b, i=i, n=n:
                                         emit_c_unit(b, i, n)))
                        if not splice:
                            pop_fillers(1e9)
                pop_fillers(1e9)
    nc.compile()
    _cache[key] = nc
    return nc


def make_in_maps(query, value, value_lens, Wq, bq, Wk, bk, Wv, bv, Wf, bf):
    query = np.ascontiguousarray(np.asarray(query, np.float32))
    value = np.ascontiguousarray(np.asarray(value, np.float32))
    value_lens = np.asarray(value_lens)
    Wq = np.asarray(Wq, np.float32)
    Wk = np.asarray(Wk, np.float32)
    Wv = np.asarray(Wv, np.float32)
    Wf = np.asarray(Wf, np.float32)
    bq = np.asarray(bq, np.float32)
    bk = np.asarray(bk, np.float32)

    scale = np.float32(1.0 / np.sqrt(np.float32(DH)))
    effL = [int(l) if l > 0 else T for l in value_lens]
    NJS = tuple(max(1, int(np.ceil(l / 128))) for l in effL)
    NSV = [(nj + 3) // 4 for nj in NJS]
    TV = [s * SW for s in NSV]
    NJT = sum(NJS)

    # batch-independent inputs: mask, stacked xq, concatenated xv
    mask = np.zeros((128, NJT), np.float32)
    joff = 0
    for b in range(B):
        L = int(value_lens[b])
        if L > 0:
            idx = np.arange(NJS[b] * 128).reshape(NJS[b], 128).T  # [128, NJ]
            mb = np.zeros((128, NJS[b]), np.float32)
            mb[idx >= L] = -LARGE
            mask[:, joff:joff + NJS[b]] = mb
        joff += NJS[b]

    xq = np.empty((B, D, T), MM_NP)
    for b in range(B):
        xq[b] = 0 if int(value_lens[b]) == 0 else query[b].T.astype(MM_NP)
    xv = np.concatenate(
        [value[b].T[:, :TV[b]].astype(MM_NP) for b in range(B)], axis=1)
    xv = np.ascontiguousarray(xv)

    in_maps = []
    for c in range(NCORES):
        cs = slice(c * CD, (c + 1) * CD)
        wq = (Wq[:, cs] * scale).astype(MM_NP)
        wk = Wk[:, cs].astype(MM_NP)
        wv = np.zeros((D, 130), np.float32)
        for h in range(2):
            wv[:, h * 65:h * 65 + 64] = Wv[:, c * CD + h * 64:
                                           c * CD + (h + 1) * 64]
        wf = Wf[cs, :].astype(MM_NP)
        bqk = np.zeros((128, 2 * B), np.float32)
        for b in range(B):
            if int(value_lens[b]) != 0:
                bqk[:, b] = bq[cs] * scale
            bqk[:, B + b] = bk[cs]
        in_maps.append({
            "xq": xq, "xv": xv,
            "wq": wq, "wk": wk, "wv": wv.astype(MM_NP), "wf": wf,
            "mask": mask, "bqk": bqk,
        })
    return in_maps, NJS


def assemble(results, Wv, bv, Wf, bf):
    bv = np.asarray(bv, np.float32)
    Wf = np.asarray(Wf, np.float32)
    bf = np.asarray(bf, np.float32)
    const = (bv @ Wf + bf).astype(np.float32)
    acc = np.zeros((B, T, ATT), np.float32)
    for r in results:
        acc += r["y"].astype(np.float32)
    return acc + const


def kernel(query, value, value_lens, Wq, bq, Wk, bk, Wv, bv, Wf, bf):
    in_maps, NJS = make_in_maps(query, value, value_lens, Wq, bq, Wk, bk,
                                Wv, bv, Wf, bf)
    nc = build_nc(NJS)
    res = run_bass_kernel_spmd(nc, in_maps, list(range(NCORES)))
    return assemble(res.results, Wv, bv, Wf, bf)
